# revision 35
# baseline (speedup 1.0000x reference)
"""Trainium2 Bass kernel for nn_BAGDnet (gnn_message_passing).

Computation (per measurement m):
    T = tKF[meas_kf[m]]          # 4x4 pose
    p = tMP[meas_mp[m]]          # 3d map point
    pts = T[:3] @ [p, 1]
    out[m] = (pts0/pts2*FX + CX, pts1/pts2*FY + CY)

idxKF / idxMP are sorted unique arange id tables, so searchsorted(idx, meas)
== meas and measurement ids index the tables directly.

Sharding strategy (data-parallel over M per the hint): 2M measurements split
across 8 cores. Per core, measurements are grouped by pose into fixed-size
cells (S=4 slots, one pose per cell, poses spanning multiple cells get their
table row duplicated), laid out as 128 partitions x 502 cells. The pose rows
are pre-projected on host into A = [FX*T0+CX*T2; FY*T1+CY*T2; T2] so the
device computes out = (A0.h/A2.h, A1.h/A2.h) with no epilogue add.

On device the pose row for a cell is never materialized per measurement:
the multiply reads the 12-value row straight from a tiny per-partition table
through a stride-0 broadcast access pattern. This cuts HBM traffic from
68 B/meas (gathered-pose streaming) to ~15 B/meas (fp16 h-vector + fp16 out
+ table), turning the kernel from DMA-bound into engine-balanced:
  DVE    : product m = A (*) h (fp16 TensorTensor 2x mode), part of the
           first adds, and the final multiply by the reciprocal
  GPSIMD : rest of the first adds + the second adds
  ACT    : the perspective reciprocal (table func, duplicated fp16 lanes)
Host gathers the points into cell order (id->row is identity here), and
un-permutes the fp16 device output back to measurement order in f32.
"""

import numpy as np

M = 2_000_000
N_KF = 2_000
N_MP = 200_000
N_CORES = 8
MC = M // N_CORES          # 250_000 measurements per core
P = 128
S = 4                      # slots per cell (one pose per cell)
CH = 502                   # cells per partition
SLOTS = CH * S             # 2008 slots per partition
TOT = P * SLOTS            # 257024 slots per core (~2.8% padding)
# small head slab starts compute sooner (first-load latency is mostly DMA
# pipeline constants + transfer); small tail slab shortens the last
# a->recip->fmul->store chain after the bulk compute ends
SLABS = [32, 224, 320, 320, 320, 320, 312, 128, 32]
assert sum(SLABS) == SLOTS and all(s % S == 0 for s in SLABS)
FX = 320.0
FY = 320.0
CX = 320.0
CY = 240.0

_CACHE = {}


def _act_recip(nc, mybir, out, in_):
    """Scalar-engine reciprocal: out = 1 / in_.

    Emitted directly (the bass wrapper refuses ActivationFunctionType.
    Reciprocal out of fp32-training accuracy caution; the act-table func is
    plenty accurate for this kernel's 2e-2 tolerance)."""
    se = nc.scalar
    ins = [se.lower_ap(in_)]
    for v in (0.0, 1.0, 0.0):      # bias, scale, alpha immediates
        ins.append(mybir.ImmediateValue(dtype=mybir.dt.float32, value=v))
    return se.add_instruction(
        mybir.InstActivation(
            name=se.bass.get_next_instruction_name(),
            func=mybir.ActivationFunctionType.Reciprocal,
            ins=ins,
            outs=[se.lower_ap(out)],
        )
    )


def _build():
    import concourse.bacc as bacc
    import concourse.mybir as mybir
    import concourse.tile as tile

    f16 = mybir.dt.float16
    f32 = mybir.dt.float32
    mult, add = mybir.AluOpType.mult, mybir.AluOpType.add

    nc = bacc.Bacc("TRN2", target_bir_lowering=False, debug=False)
    s0, c0 = SLABS[0], SLABS[0] // S
    PRE = c0 * 12 + s0 * 5     # slab-0 table + h packed into one tensor so
    hp = nc.dram_tensor("hp", [P, SLOTS * 5], f16, kind="ExternalInput")
    tb = nc.dram_tensor("tb", [P, CH * 12], f16, kind="ExternalInput")
    pre = nc.dram_tensor("pre", [P, PRE], f16, kind="ExternalInput")
    ot = nc.dram_tensor("ot", [P, SLOTS * 2], f16, kind="ExternalOutput")

    with tile.TileContext(nc) as tc:
        with tc.tile_pool(name="hpool", bufs=4) as hpool, \
             tc.tile_pool(name="tpool", bufs=4) as tpool, \
             tc.tile_pool(name="mpool", bufs=3) as mpool, \
             tc.tile_pool(name="spool", bufs=3) as spool, \
             tc.tile_pool(name="apool", bufs=3) as apool, \
             tc.tile_pool(name="opool", bufs=3) as opool:
            def head(o, sls, so):
                """Slab front: loads, products, 2x-mode adds. Returns state."""
                chs = sls // S
                co = so // S
                ld_a = nc.sync
                ld_b = nc.sync
                if o == 0:
                    # slab 0: table + h arrive in ONE load (the first product
                    # is gated by this; a second serialized HWDGE slot would
                    # cost ~0.6us of pipeline fill)
                    ct = hpool.tile([P, PRE], f16, tag="ct")
                    ld_a.dma_start(out=ct[:], in_=pre.ap()[:, :])
                    tt_ap = ct[:][:, 0:chs * 12]
                    ht_ap = ct[:][:, chs * 12:PRE]
                else:
                    ht = hpool.tile([P, sls * 5], f16, tag="ht")
                    tt = tpool.tile([P, chs * 12], f16, tag="tt")
                    # table first: smaller transfer, and it serializes behind
                    # the h load on the shared HWDGE otherwise
                    ld_b.dma_start(out=tt[:],
                                   in_=tb.ap()[:, co * 12:(co + chs) * 12])
                    ld_a.dma_start(out=ht[:],
                                   in_=hp.ap()[:, so * 5:(so + sls) * 5])
                    tt_ap = tt[:]
                    ht_ap = ht[:]
                # h stream per slot is (x, y, z, z, z); table row per cell is
                # [A(i,j01) pairs (6)] [A(i,2) (3)] [A(i,3) (3)] so that every
                # operand below has a packed (stride-1) last dim -> DVE 2x.
                # mA[p, cell, s, i, k] = A[cell, i, k] * h[cell, s, k], k=x,y
                mA = mpool.tile([P, sls * 6], f16, tag="mA")
                t_pair = tt_ap.rearrange("p (seg o i k) -> p seg o i k",
                                         seg=chs, o=1, i=6, k=2)[:, :, :, 0:3] \
                              .to_broadcast([P, chs, S, 3, 2])
                h_xy = ht_ap.rearrange("p (seg s o c) -> p seg s o c",
                                       seg=chs, s=S, o=1, c=5)[:, :, :, :, 0:2] \
                            .to_broadcast([P, chs, S, 3, 2])
                mA_v = mA[:].rearrange("p (seg s i k) -> p seg s i k",
                                       seg=chs, s=S, i=3, k=2)
                nc.vector.tensor_tensor(out=mA_v, in0=h_xy, in1=t_pair, op=mult)
                # mB[p, cell, s, i] = A[cell, i, 2] * z   (z replicated in h)
                mB = mpool.tile([P, sls * 3], f16, tag="mB")
                t_z = tt_ap.rearrange("p (seg o i) -> p seg o i",
                                      seg=chs, o=1, i=12)[:, :, :, 6:9] \
                           .to_broadcast([P, chs, S, 3])
                h_zzz = ht_ap.rearrange("p (seg s c) -> p seg s c",
                                        seg=chs, s=S, c=5)[:, :, :, 2:5]
                mB_v = mB[:].rearrange("p (seg s i) -> p seg s i",
                                       seg=chs, s=S, i=3)
                nc.vector.tensor_tensor(out=mB_v, in0=h_zzz, in1=t_z, op=mult)
                # s1b[p, sl, i] = mB + A[cell, i, 3]   (translation fold, 2x)
                s1b = spool.tile([P, sls * 3], f16, tag="s1b")
                s1b_v = s1b[:].rearrange("p (seg s i) -> p seg s i",
                                         seg=chs, s=S, i=3)
                t_c = tt_ap.rearrange("p (seg o i) -> p seg o i",
                                      seg=chs, o=1, i=12)[:, :, :, 9:12] \
                           .to_broadcast([P, chs, S, 3])
                nc.vector.tensor_tensor(out=s1b_v, in0=mB_v, in1=t_c, op=add)
                # s1a[p, sl, i] = mA[.., 0] + mA[.., 1]  (strided; gpsimd)
                mAs = mA[:].rearrange("p (sl i k) -> p sl i k", i=3, k=2)
                s1a = spool.tile([P, sls * 3], f16, tag="s1a")
                s1a_v = s1a[:].rearrange("p (sl i) -> p sl i", i=3)
                nc.gpsimd.tensor_tensor(out=s1a_v, in0=mAs[:, :, :, 0],
                                        in1=mAs[:, :, :, 1], op=add)
                return (sls, so, s1a[:].rearrange("p (sl i) -> p sl i", i=3),
                        s1b[:].rearrange("p (sl i) -> p sl i", i=3), ld_b)

            def mid(st):
                """Slab middle: final adds (split DVE/gpsimd) + reciprocal."""
                sls, so, s1a_v, s1b_v, ld_b = st
                k2 = (sls * 39 // 64) // 4 * 4   # a slots on DVE, rest gpsimd
                # a[p, sl, i] = s1a + s1b   (both packed fp16 -> DVE 2x)
                a = apool.tile([P, sls * 3], f16, tag="a")
                av = a[:].rearrange("p (sl i) -> p sl i", i=3)
                nc.vector.tensor_tensor(out=av[:, 0:k2], in0=s1a_v[:, 0:k2],
                                        in1=s1b_v[:, 0:k2], op=add)
                nc.gpsimd.tensor_tensor(out=av[:, k2:sls], in0=s1a_v[:, k2:sls],
                                        in1=s1b_v[:, k2:sls], op=add)
                # rzh[p, sl, c] = 1/a2 duplicated into two packed fp16 lanes
                # (scalar engine reciprocal; interp-exact, z in [3,7] is well
                # inside the +-[2^-42, 2^42] valid range)
                rzh = apool.tile([P, sls * 2], f16, tag="rzh")
                rzhv = rzh[:].rearrange("p (sl c) -> p sl c", c=2)
                _act_recip(nc, mybir, out=rzhv,
                           in_=av[:, :, 2:3].to_broadcast([P, sls, 2]))
                return (sls, so, av, rzhv, ld_b)

            def tail(st):
                """Slab back: perspective multiply + store."""
                sls, so, av, rzhv, ld_b = st
                # out = a01 * rzh   (DVE, 2x: all packed fp16)
                otile = opool.tile([P, sls * 2], f16, tag="ot")
                ov = otile[:].rearrange("p (sl c) -> p sl c", c=2)
                nc.vector.tensor_tensor(out=ov, in0=av[:, :, 0:2], in1=rzhv,
                                        op=mult)
                ld_b.dma_start(out=ot.ap()[:, so * 2:(so + sls) * 2],
                               in_=otile[:])

            # 2-stage software pipeline: the mult->s1->a->recip->fmul chain
            # spans more than one slab period, so slab o's mid runs after
            # head[o+1] and its tail after head[o+2] — no in-order engine
            # queue ever waits across slabs
            stages = []
            so = 0
            for o, sls in enumerate(SLABS):
                stages.append(head(o, sls, so))
                if o >= 1:
                    stages[o - 1] = mid(stages[o - 1])
                if o >= 2:
                    tail(stages[o - 2])
                so += sls
            n = len(SLABS)
            stages[n - 1] = mid(stages[n - 1])
            tail(stages[n - 2])
            tail(stages[n - 1])
    nc.compile()
    return nc


def get_nc():
    if "nc" not in _CACHE:
        _CACHE["nc"] = _build()
    return _CACHE["nc"]


def make_in_maps(tMP, tKF, kf_rows, mp_rows):
    """Pack measurements into pose-cells; returns per-core inputs + slot maps."""
    T = np.asarray(tKF, dtype=np.float32)
    A = np.empty((N_KF, 3, 4), np.float32)
    A[:, 0] = FX * T[:, 0] + CX * T[:, 2]
    A[:, 1] = FY * T[:, 1] + CY * T[:, 2]
    A[:, 2] = T[:, 2]
    # row layout per cell: [A(i,j01) pairs i-major (6)] [A(:,2) (3)] [A(:,3) (3)]
    A12 = np.concatenate([A[:, :, 0:2].reshape(N_KF, 6),
                          A[:, :, 2], A[:, :, 3]], axis=1).astype(np.float16)
    empty_row = np.zeros(12, np.float16)
    empty_row[11] = 1.0        # a2 = 1 for padding cells -> out = 0, no NaN
    tMP = np.asarray(tMP, dtype=np.float32)
    # h stream per slot: (x, y, z, z, z) - z replicated so the z-products
    # read a packed lane per i
    homo = np.empty((N_MP, 5), np.float32)
    homo[:, 0:2] = tMP[:, 0:2]
    homo[:, 2:5] = tMP[:, 2:3]
    homo = homo.astype(np.float16)
    in_maps = []
    slot_maps = []
    for c in range(N_CORES):
        kf = kf_rows[c * MC:(c + 1) * MC]
        mp = mp_rows[c * MC:(c + 1) * MC]
        counts = np.bincount(kf, minlength=N_KF)
        ncells_k = -(-counts // S)
        cell_off = np.concatenate([[0], np.cumsum(ncells_k)])
        ncells = int(cell_off[-1])
        assert ncells <= P * CH, f"cell overflow: {ncells} > {P * CH}"
        order = np.argsort(kf, kind="stable")
        kfs = kf[order]
        starts = np.concatenate([[0], np.cumsum(counts)])
        j = np.arange(MC, dtype=np.int64) - starts[kfs]
        slot = (cell_off[kfs] + j // S) * S + (j % S)    # flat in [0, TOT)
        hpa = np.zeros((TOT, 5), np.float16)
        hpa[slot] = homo[mp[order]]
        kcell = np.repeat(np.arange(N_KF), ncells_k)
        tbl = np.empty((P * CH, 12), np.float16)
        tbl[:ncells] = A12[kcell]
        tbl[ncells:] = empty_row
        # slab-0 prefetch: its table cells + h slots packed per partition
        s0, c0 = SLABS[0], SLABS[0] // S
        tbl_p = tbl.reshape(P, CH, 12)
        hpa_p = hpa.reshape(P, SLOTS, 5)
        prea = np.concatenate([tbl_p[:, 0:c0].reshape(P, c0 * 12),
                               hpa_p[:, 0:s0].reshape(P, s0 * 5)], axis=1)
        in_maps.append({"hp": hpa.reshape(P, SLOTS * 5),
                        "tb": tbl.reshape(P, CH * 12),
                        "pre": prea})
        slot_maps.append((order, slot))
    return in_maps, slot_maps


def assemble(results, slot_maps):
    outs = []
    for c in range(N_CORES):
        o = np.asarray(results[c]["ot"]).reshape(TOT, 2)
        order, slot = slot_maps[c]
        r = np.empty((MC, 2), np.float32)
        r[order] = o[slot].astype(np.float32)
        outs.append(r)
    return np.concatenate(outs, axis=0)


def kernel(tMP, tKF, idxKF, idxMP, meas_kf, meas_mp):
    import time

    from concourse.bass_utils import run_bass_kernel_spmd

    nc = get_nc()
    # id -> row resolution (identity for sorted arange id tables)
    kf_rows = np.searchsorted(np.asarray(idxKF), np.asarray(meas_kf)).astype(np.int64)
    mp_rows = np.searchsorted(np.asarray(idxMP), np.asarray(meas_mp)).astype(np.int64)
    in_maps, slot_maps = make_in_maps(np.asarray(tMP), np.asarray(tKF),
                                      kf_rows, mp_rows)
    try:
        res = run_bass_kernel_spmd(nc, in_maps, core_ids=list(range(N_CORES)))
    except Exception:
        # transient NRT exec-unit errors have been observed when a previous
        # process was still draining the cores; one retry recovers them
        time.sleep(2.0)
        res = run_bass_kernel_spmd(nc, in_maps, core_ids=list(range(N_CORES)))
    return assemble(res.results, slot_maps)


# revision 36
# speedup vs baseline: 1.0233x; 1.0233x over previous
"""Trainium2 Bass kernel for nn_BAGDnet (gnn_message_passing).

Computation (per measurement m):
    T = tKF[meas_kf[m]]          # 4x4 pose
    p = tMP[meas_mp[m]]          # 3d map point
    pts = T[:3] @ [p, 1]
    out[m] = (pts0/pts2*FX + CX, pts1/pts2*FY + CY)

idxKF / idxMP are sorted unique arange id tables, so searchsorted(idx, meas)
== meas and measurement ids index the tables directly.

Sharding strategy (data-parallel over M per the hint): 2M measurements split
across 8 cores. Per core, measurements are grouped by pose into fixed-size
cells (S=4 slots, one pose per cell, poses spanning multiple cells get their
table row duplicated), laid out as 128 partitions x 502 cells. The pose rows
are pre-projected on host into A = [FX*T0+CX*T2; FY*T1+CY*T2; T2] so the
device computes out = (A0.h/A2.h, A1.h/A2.h) with no epilogue add.

On device the pose row for a cell is never materialized per measurement:
the multiply reads the 12-value row straight from a tiny per-partition table
through a stride-0 broadcast access pattern. This cuts HBM traffic from
68 B/meas (gathered-pose streaming) to ~15 B/meas (fp16 h-vector + fp16 out
+ table), turning the kernel from DMA-bound into engine-balanced:
  DVE    : product m = A (*) h (fp16 TensorTensor 2x mode), part of the
           first adds, and the final multiply by the reciprocal
  GPSIMD : rest of the first adds + the second adds
  ACT    : the perspective reciprocal (table func, duplicated fp16 lanes)
Host gathers the points into cell order (id->row is identity here), and
un-permutes the fp16 device output back to measurement order in f32.
"""

import numpy as np

M = 2_000_000
N_KF = 2_000
N_MP = 200_000
N_CORES = 8
MC = M // N_CORES          # 250_000 measurements per core
P = 128
S = 4                      # slots per cell (one pose per cell)
CH = 502                   # cells per partition
SLOTS = CH * S             # 2008 slots per partition
TOT = P * SLOTS            # 257024 slots per core (~2.8% padding)
# small head slab starts compute sooner (first-load latency is mostly DMA
# pipeline constants + transfer); small tail slab shortens the last
# a->recip->fmul->store chain after the bulk compute ends
SLABS = [64, 192, 288, 320, 320, 320, 312, 128, 64]
assert sum(SLABS) == SLOTS and all(s % S == 0 for s in SLABS)
FX = 320.0
FY = 320.0
CX = 320.0
CY = 240.0

_CACHE = {}


def _act_recip(nc, mybir, out, in_):
    """Scalar-engine reciprocal: out = 1 / in_.

    Emitted directly (the bass wrapper refuses ActivationFunctionType.
    Reciprocal out of fp32-training accuracy caution; the act-table func is
    plenty accurate for this kernel's 2e-2 tolerance)."""
    se = nc.scalar
    ins = [se.lower_ap(in_)]
    for v in (0.0, 1.0, 0.0):      # bias, scale, alpha immediates
        ins.append(mybir.ImmediateValue(dtype=mybir.dt.float32, value=v))
    return se.add_instruction(
        mybir.InstActivation(
            name=se.bass.get_next_instruction_name(),
            func=mybir.ActivationFunctionType.Reciprocal,
            ins=ins,
            outs=[se.lower_ap(out)],
        )
    )


def _build():
    import concourse.bacc as bacc
    import concourse.mybir as mybir
    import concourse.tile as tile

    f16 = mybir.dt.float16
    f32 = mybir.dt.float32
    mult, add = mybir.AluOpType.mult, mybir.AluOpType.add

    nc = bacc.Bacc("TRN2", target_bir_lowering=False, debug=False)
    s0, c0 = SLABS[0], SLABS[0] // S
    PRE = c0 * 12 + s0 * 5     # slab-0 table + h packed into one tensor so
    hp = nc.dram_tensor("hp", [P, SLOTS * 5], f16, kind="ExternalInput")
    tb = nc.dram_tensor("tb", [P, CH * 12], f16, kind="ExternalInput")
    pre = nc.dram_tensor("pre", [P, PRE], f16, kind="ExternalInput")
    ot = nc.dram_tensor("ot", [P, SLOTS * 2], f16, kind="ExternalOutput")

    with tile.TileContext(nc) as tc:
        with tc.tile_pool(name="hpool", bufs=4) as hpool, \
             tc.tile_pool(name="tpool", bufs=4) as tpool, \
             tc.tile_pool(name="mpool", bufs=4) as mpool, \
             tc.tile_pool(name="spool", bufs=4) as spool, \
             tc.tile_pool(name="apool", bufs=3) as apool, \
             tc.tile_pool(name="opool", bufs=3) as opool:
            def head(o, sls, so):
                """Slab front: loads, products, 2x-mode adds. Returns state."""
                chs = sls // S
                co = so // S
                ld_a = nc.sync
                ld_b = nc.sync
                if o == 0:
                    # slab 0: table + h arrive in ONE load (the first product
                    # is gated by this; a second serialized HWDGE slot would
                    # cost ~0.6us of pipeline fill)
                    ct = hpool.tile([P, PRE], f16, tag="ct")
                    ld_a.dma_start(out=ct[:], in_=pre.ap()[:, :])
                    tt_ap = ct[:][:, 0:chs * 12]
                    ht_ap = ct[:][:, chs * 12:PRE]
                else:
                    ht = hpool.tile([P, sls * 5], f16, tag="ht")
                    tt = tpool.tile([P, chs * 12], f16, tag="tt")
                    # table first: smaller transfer, and it serializes behind
                    # the h load on the shared HWDGE otherwise
                    ld_b.dma_start(out=tt[:],
                                   in_=tb.ap()[:, co * 12:(co + chs) * 12])
                    ld_a.dma_start(out=ht[:],
                                   in_=hp.ap()[:, so * 5:(so + sls) * 5])
                    tt_ap = tt[:]
                    ht_ap = ht[:]
                # h stream per slot is (x, y, z, z, z); table row per cell is
                # [A(i,j01) pairs (6)] [A(i,2) (3)] [A(i,3) (3)] so that every
                # operand below has a packed (stride-1) last dim -> DVE 2x.
                # mA[p, cell, s, i, k] = A[cell, i, k] * h[cell, s, k], k=x,y
                mA = mpool.tile([P, sls * 6], f16, tag="mA")
                t_pair = tt_ap.rearrange("p (seg o i k) -> p seg o i k",
                                         seg=chs, o=1, i=6, k=2)[:, :, :, 0:3] \
                              .to_broadcast([P, chs, S, 3, 2])
                h_xy = ht_ap.rearrange("p (seg s o c) -> p seg s o c",
                                       seg=chs, s=S, o=1, c=5)[:, :, :, :, 0:2] \
                            .to_broadcast([P, chs, S, 3, 2])
                mA_v = mA[:].rearrange("p (seg s i k) -> p seg s i k",
                                       seg=chs, s=S, i=3, k=2)
                nc.vector.tensor_tensor(out=mA_v, in0=h_xy, in1=t_pair, op=mult)
                # mB[p, cell, s, i] = A[cell, i, 2] * z   (z replicated in h)
                mB = mpool.tile([P, sls * 3], f16, tag="mB")
                t_z = tt_ap.rearrange("p (seg o i) -> p seg o i",
                                      seg=chs, o=1, i=12)[:, :, :, 6:9] \
                           .to_broadcast([P, chs, S, 3])
                h_zzz = ht_ap.rearrange("p (seg s c) -> p seg s c",
                                        seg=chs, s=S, c=5)[:, :, :, 2:5]
                mB_v = mB[:].rearrange("p (seg s i) -> p seg s i",
                                       seg=chs, s=S, i=3)
                nc.vector.tensor_tensor(out=mB_v, in0=h_zzz, in1=t_z, op=mult)
                # s1b[p, sl, i] = mB + A[cell, i, 3]   (translation fold, 2x)
                s1b = spool.tile([P, sls * 3], f16, tag="s1b")
                s1b_v = s1b[:].rearrange("p (seg s i) -> p seg s i",
                                         seg=chs, s=S, i=3)
                t_c = tt_ap.rearrange("p (seg o i) -> p seg o i",
                                      seg=chs, o=1, i=12)[:, :, :, 9:12] \
                           .to_broadcast([P, chs, S, 3])
                nc.vector.tensor_tensor(out=s1b_v, in0=mB_v, in1=t_c, op=add)
                # s1a[p, sl, i] = mA[.., 0] + mA[.., 1]  (strided; gpsimd)
                mAs = mA[:].rearrange("p (sl i k) -> p sl i k", i=3, k=2)
                s1a = spool.tile([P, sls * 3], f16, tag="s1a")
                s1a_v = s1a[:].rearrange("p (sl i) -> p sl i", i=3)
                nc.gpsimd.tensor_tensor(out=s1a_v, in0=mAs[:, :, :, 0],
                                        in1=mAs[:, :, :, 1], op=add)
                return (sls, so, s1a[:].rearrange("p (sl i) -> p sl i", i=3),
                        s1b[:].rearrange("p (sl i) -> p sl i", i=3), ld_b)

            def mid(st):
                """Slab middle: final adds (split DVE/gpsimd) + reciprocal."""
                sls, so, s1a_v, s1b_v, ld_b = st
                k2 = (sls * 39 // 64) // 4 * 4   # a slots on DVE, rest gpsimd
                # a[p, sl, i] = s1a + s1b   (both packed fp16 -> DVE 2x)
                a = apool.tile([P, sls * 3], f16, tag="a")
                av = a[:].rearrange("p (sl i) -> p sl i", i=3)
                nc.vector.tensor_tensor(out=av[:, 0:k2], in0=s1a_v[:, 0:k2],
                                        in1=s1b_v[:, 0:k2], op=add)
                nc.gpsimd.tensor_tensor(out=av[:, k2:sls], in0=s1a_v[:, k2:sls],
                                        in1=s1b_v[:, k2:sls], op=add)
                # rzh[p, sl, c] = 1/a2 duplicated into two packed fp16 lanes
                # (scalar engine reciprocal; interp-exact, z in [3,7] is well
                # inside the +-[2^-42, 2^42] valid range)
                rzh = apool.tile([P, sls * 2], f16, tag="rzh")
                rzhv = rzh[:].rearrange("p (sl c) -> p sl c", c=2)
                _act_recip(nc, mybir, out=rzhv,
                           in_=av[:, :, 2:3].to_broadcast([P, sls, 2]))
                return (sls, so, av, rzhv, ld_b)

            def tail(st):
                """Slab back: perspective multiply + store."""
                sls, so, av, rzhv, ld_b = st
                # out = a01 * rzh   (DVE, 2x: all packed fp16)
                otile = opool.tile([P, sls * 2], f16, tag="ot")
                ov = otile[:].rearrange("p (sl c) -> p sl c", c=2)
                nc.vector.tensor_tensor(out=ov, in0=av[:, :, 0:2], in1=rzhv,
                                        op=mult)
                ld_b.dma_start(out=ot.ap()[:, so * 2:(so + sls) * 2],
                               in_=otile[:])

            # 2-stage software pipeline: the mult->s1->a->recip->fmul chain
            # spans more than one slab period, so slab o's mid runs after
            # head[o+1] and its tail after head[o+2] — no in-order engine
            # queue ever waits across slabs
            stages = []
            so = 0
            for o, sls in enumerate(SLABS):
                stages.append(head(o, sls, so))
                if o >= 1:
                    stages[o - 1] = mid(stages[o - 1])
                if o >= 2:
                    tail(stages[o - 2])
                so += sls
            n = len(SLABS)
            stages[n - 1] = mid(stages[n - 1])
            tail(stages[n - 2])
            tail(stages[n - 1])
    nc.compile()
    return nc


def get_nc():
    if "nc" not in _CACHE:
        _CACHE["nc"] = _build()
    return _CACHE["nc"]


def make_in_maps(tMP, tKF, kf_rows, mp_rows):
    """Pack measurements into pose-cells; returns per-core inputs + slot maps."""
    T = np.asarray(tKF, dtype=np.float32)
    A = np.empty((N_KF, 3, 4), np.float32)
    A[:, 0] = FX * T[:, 0] + CX * T[:, 2]
    A[:, 1] = FY * T[:, 1] + CY * T[:, 2]
    A[:, 2] = T[:, 2]
    # row layout per cell: [A(i,j01) pairs i-major (6)] [A(:,2) (3)] [A(:,3) (3)]
    A12 = np.concatenate([A[:, :, 0:2].reshape(N_KF, 6),
                          A[:, :, 2], A[:, :, 3]], axis=1).astype(np.float16)
    empty_row = np.zeros(12, np.float16)
    empty_row[11] = 1.0        # a2 = 1 for padding cells -> out = 0, no NaN
    tMP = np.asarray(tMP, dtype=np.float32)
    # h stream per slot: (x, y, z, z, z) - z replicated so the z-products
    # read a packed lane per i
    homo = np.empty((N_MP, 5), np.float32)
    homo[:, 0:2] = tMP[:, 0:2]
    homo[:, 2:5] = tMP[:, 2:3]
    homo = homo.astype(np.float16)
    in_maps = []
    slot_maps = []
    for c in range(N_CORES):
        kf = kf_rows[c * MC:(c + 1) * MC]
        mp = mp_rows[c * MC:(c + 1) * MC]
        counts = np.bincount(kf, minlength=N_KF)
        ncells_k = -(-counts // S)
        cell_off = np.concatenate([[0], np.cumsum(ncells_k)])
        ncells = int(cell_off[-1])
        assert ncells <= P * CH, f"cell overflow: {ncells} > {P * CH}"
        order = np.argsort(kf, kind="stable")
        kfs = kf[order]
        starts = np.concatenate([[0], np.cumsum(counts)])
        j = np.arange(MC, dtype=np.int64) - starts[kfs]
        slot = (cell_off[kfs] + j // S) * S + (j % S)    # flat in [0, TOT)
        hpa = np.zeros((TOT, 5), np.float16)
        hpa[slot] = homo[mp[order]]
        kcell = np.repeat(np.arange(N_KF), ncells_k)
        tbl = np.empty((P * CH, 12), np.float16)
        tbl[:ncells] = A12[kcell]
        tbl[ncells:] = empty_row
        # slab-0 prefetch: its table cells + h slots packed per partition
        s0, c0 = SLABS[0], SLABS[0] // S
        tbl_p = tbl.reshape(P, CH, 12)
        hpa_p = hpa.reshape(P, SLOTS, 5)
        prea = np.concatenate([tbl_p[:, 0:c0].reshape(P, c0 * 12),
                               hpa_p[:, 0:s0].reshape(P, s0 * 5)], axis=1)
        in_maps.append({"hp": hpa.reshape(P, SLOTS * 5),
                        "tb": tbl.reshape(P, CH * 12),
                        "pre": prea})
        slot_maps.append((order, slot))
    return in_maps, slot_maps


def assemble(results, slot_maps):
    outs = []
    for c in range(N_CORES):
        o = np.asarray(results[c]["ot"]).reshape(TOT, 2)
        order, slot = slot_maps[c]
        r = np.empty((MC, 2), np.float32)
        r[order] = o[slot].astype(np.float32)
        outs.append(r)
    return np.concatenate(outs, axis=0)


def kernel(tMP, tKF, idxKF, idxMP, meas_kf, meas_mp):
    import time

    from concourse.bass_utils import run_bass_kernel_spmd

    nc = get_nc()
    # id -> row resolution (identity for sorted arange id tables)
    kf_rows = np.searchsorted(np.asarray(idxKF), np.asarray(meas_kf)).astype(np.int64)
    mp_rows = np.searchsorted(np.asarray(idxMP), np.asarray(meas_mp)).astype(np.int64)
    in_maps, slot_maps = make_in_maps(np.asarray(tMP), np.asarray(tKF),
                                      kf_rows, mp_rows)
    try:
        res = run_bass_kernel_spmd(nc, in_maps, core_ids=list(range(N_CORES)))
    except Exception:
        # transient NRT exec-unit errors have been observed when a previous
        # process was still draining the cores; one retry recovers them
        time.sleep(2.0)
        res = run_bass_kernel_spmd(nc, in_maps, core_ids=list(range(N_CORES)))
    return assemble(res.results, slot_maps)


# revision 37
# speedup vs baseline: 1.0356x; 1.0119x over previous
"""Trainium2 Bass kernel for nn_BAGDnet (gnn_message_passing).

Computation (per measurement m):
    T = tKF[meas_kf[m]]          # 4x4 pose
    p = tMP[meas_mp[m]]          # 3d map point
    pts = T[:3] @ [p, 1]
    out[m] = (pts0/pts2*FX + CX, pts1/pts2*FY + CY)

idxKF / idxMP are sorted unique arange id tables, so searchsorted(idx, meas)
== meas and measurement ids index the tables directly.

Sharding strategy (data-parallel over M per the hint): 2M measurements split
across 8 cores. Per core, measurements are grouped by pose into fixed-size
cells (S=4 slots, one pose per cell, poses spanning multiple cells get their
table row duplicated), laid out as 128 partitions x 502 cells. The pose rows
are pre-projected on host into A = [FX*T0+CX*T2; FY*T1+CY*T2; T2] so the
device computes out = (A0.h/A2.h, A1.h/A2.h) with no epilogue add.

On device the pose row for a cell is never materialized per measurement:
the multiply reads the 12-value row straight from a tiny per-partition table
through a stride-0 broadcast access pattern. This cuts HBM traffic from
68 B/meas (gathered-pose streaming) to ~15 B/meas (fp16 h-vector + fp16 out
+ table), turning the kernel from DMA-bound into engine-balanced:
  DVE    : product m = A (*) h (fp16 TensorTensor 2x mode), part of the
           first adds, and the final multiply by the reciprocal
  GPSIMD : rest of the first adds + the second adds
  ACT    : the perspective reciprocal (table func, duplicated fp16 lanes)
Host gathers the points into cell order (id->row is identity here), and
un-permutes the fp16 device output back to measurement order in f32.
"""

import numpy as np

M = 2_000_000
N_KF = 2_000
N_MP = 200_000
N_CORES = 8
MC = M // N_CORES          # 250_000 measurements per core
P = 128
S = 4                      # slots per cell (one pose per cell)
CH = 502                   # cells per partition
SLOTS = CH * S             # 2008 slots per partition
TOT = P * SLOTS            # 257024 slots per core (~2.8% padding)
# small head slab starts compute sooner (first-load latency is mostly DMA
# pipeline constants + transfer); small tail slab shortens the last
# a->recip->fmul->store chain after the bulk compute ends
SLABS = [64, 192, 288, 320, 320, 320, 312, 128, 64]
assert sum(SLABS) == SLOTS and all(s % S == 0 for s in SLABS)
FX = 320.0
FY = 320.0
CX = 320.0
CY = 240.0

_CACHE = {}


def _act_recip(nc, mybir, out, in_):
    """Scalar-engine reciprocal: out = 1 / in_.

    Emitted directly (the bass wrapper refuses ActivationFunctionType.
    Reciprocal out of fp32-training accuracy caution; the act-table func is
    plenty accurate for this kernel's 2e-2 tolerance)."""
    se = nc.scalar
    ins = [se.lower_ap(in_)]
    for v in (0.0, 1.0, 0.0):      # bias, scale, alpha immediates
        ins.append(mybir.ImmediateValue(dtype=mybir.dt.float32, value=v))
    return se.add_instruction(
        mybir.InstActivation(
            name=se.bass.get_next_instruction_name(),
            func=mybir.ActivationFunctionType.Reciprocal,
            ins=ins,
            outs=[se.lower_ap(out)],
        )
    )


def _build():
    import concourse.bacc as bacc
    import concourse.mybir as mybir
    import concourse.tile as tile

    f16 = mybir.dt.float16
    f32 = mybir.dt.float32
    mult, add = mybir.AluOpType.mult, mybir.AluOpType.add

    nc = bacc.Bacc("TRN2", target_bir_lowering=False, debug=False)
    # per-slab [table-cells | h-slots] packed into ONE stream: one DMA per
    # slab (instead of two) halves the serialized HWDGE issue slots
    TOTB = CH * 12 + SLOTS * 5
    hb = nc.dram_tensor("hb", [P, TOTB], f16, kind="ExternalInput")
    ot = nc.dram_tensor("ot", [P, SLOTS * 2], f16, kind="ExternalOutput")

    with tile.TileContext(nc) as tc:
        with tc.tile_pool(name="hpool", bufs=4) as hpool, \
             tc.tile_pool(name="tpool", bufs=4) as tpool, \
             tc.tile_pool(name="mpool", bufs=4) as mpool, \
             tc.tile_pool(name="spool", bufs=4) as spool, \
             tc.tile_pool(name="apool", bufs=3) as apool, \
             tc.tile_pool(name="opool", bufs=3) as opool:
            def head(o, sls, so):
                """Slab front: loads, products, 2x-mode adds. Returns state."""
                chs = sls // S
                bo = so * 5 + (so // S) * 12     # slab offset in hb
                sz = chs * 12 + sls * 5
                ld_b = nc.sync
                ct = hpool.tile([P, sz], f16, tag="ct")
                ld_b.dma_start(out=ct[:], in_=hb.ap()[:, bo:bo + sz])
                tt_ap = ct[:][:, 0:chs * 12]
                ht_ap = ct[:][:, chs * 12:sz]
                # h stream per slot is (x, y, z, z, z); table row per cell is
                # [A(i,j01) pairs (6)] [A(i,2) (3)] [A(i,3) (3)] so that every
                # operand below has a packed (stride-1) last dim -> DVE 2x.
                # mA[p, cell, s, i, k] = A[cell, i, k] * h[cell, s, k], k=x,y
                mA = mpool.tile([P, sls * 6], f16, tag="mA")
                t_pair = tt_ap.rearrange("p (seg o i k) -> p seg o i k",
                                         seg=chs, o=1, i=6, k=2)[:, :, :, 0:3] \
                              .to_broadcast([P, chs, S, 3, 2])
                h_xy = ht_ap.rearrange("p (seg s o c) -> p seg s o c",
                                       seg=chs, s=S, o=1, c=5)[:, :, :, :, 0:2] \
                            .to_broadcast([P, chs, S, 3, 2])
                mA_v = mA[:].rearrange("p (seg s i k) -> p seg s i k",
                                       seg=chs, s=S, i=3, k=2)
                nc.vector.tensor_tensor(out=mA_v, in0=h_xy, in1=t_pair, op=mult)
                # mB[p, cell, s, i] = A[cell, i, 2] * z   (z replicated in h)
                mB = mpool.tile([P, sls * 3], f16, tag="mB")
                t_z = tt_ap.rearrange("p (seg o i) -> p seg o i",
                                      seg=chs, o=1, i=12)[:, :, :, 6:9] \
                           .to_broadcast([P, chs, S, 3])
                h_zzz = ht_ap.rearrange("p (seg s c) -> p seg s c",
                                        seg=chs, s=S, c=5)[:, :, :, 2:5]
                mB_v = mB[:].rearrange("p (seg s i) -> p seg s i",
                                       seg=chs, s=S, i=3)
                nc.vector.tensor_tensor(out=mB_v, in0=h_zzz, in1=t_z, op=mult)
                # s1b[p, sl, i] = mB + A[cell, i, 3]   (translation fold, 2x)
                s1b = spool.tile([P, sls * 3], f16, tag="s1b")
                s1b_v = s1b[:].rearrange("p (seg s i) -> p seg s i",
                                         seg=chs, s=S, i=3)
                t_c = tt_ap.rearrange("p (seg o i) -> p seg o i",
                                      seg=chs, o=1, i=12)[:, :, :, 9:12] \
                           .to_broadcast([P, chs, S, 3])
                nc.vector.tensor_tensor(out=s1b_v, in0=mB_v, in1=t_c, op=add)
                # s1a[p, sl, i] = mA[.., 0] + mA[.., 1]  (strided; gpsimd)
                mAs = mA[:].rearrange("p (sl i k) -> p sl i k", i=3, k=2)
                s1a = spool.tile([P, sls * 3], f16, tag="s1a")
                s1a_v = s1a[:].rearrange("p (sl i) -> p sl i", i=3)
                nc.gpsimd.tensor_tensor(out=s1a_v, in0=mAs[:, :, :, 0],
                                        in1=mAs[:, :, :, 1], op=add)
                return (sls, so, s1a[:].rearrange("p (sl i) -> p sl i", i=3),
                        s1b[:].rearrange("p (sl i) -> p sl i", i=3), ld_b)

            def mid(st):
                """Slab middle: final adds (split DVE/gpsimd) + reciprocal."""
                sls, so, s1a_v, s1b_v, ld_b = st
                k2 = (sls * 39 // 64) // 4 * 4   # a slots on DVE, rest gpsimd
                # a[p, sl, i] = s1a + s1b   (both packed fp16 -> DVE 2x)
                a = apool.tile([P, sls * 3], f16, tag="a")
                av = a[:].rearrange("p (sl i) -> p sl i", i=3)
                nc.vector.tensor_tensor(out=av[:, 0:k2], in0=s1a_v[:, 0:k2],
                                        in1=s1b_v[:, 0:k2], op=add)
                nc.gpsimd.tensor_tensor(out=av[:, k2:sls], in0=s1a_v[:, k2:sls],
                                        in1=s1b_v[:, k2:sls], op=add)
                # rzh[p, sl, c] = 1/a2 duplicated into two packed fp16 lanes
                # (scalar engine reciprocal; interp-exact, z in [3,7] is well
                # inside the +-[2^-42, 2^42] valid range)
                rzh = apool.tile([P, sls * 2], f16, tag="rzh")
                rzhv = rzh[:].rearrange("p (sl c) -> p sl c", c=2)
                _act_recip(nc, mybir, out=rzhv,
                           in_=av[:, :, 2:3].to_broadcast([P, sls, 2]))
                return (sls, so, av, rzhv, ld_b)

            def tail(st):
                """Slab back: perspective multiply + store."""
                sls, so, av, rzhv, ld_b = st
                # out = a01 * rzh   (DVE, 2x: all packed fp16)
                otile = opool.tile([P, sls * 2], f16, tag="ot")
                ov = otile[:].rearrange("p (sl c) -> p sl c", c=2)
                nc.vector.tensor_tensor(out=ov, in0=av[:, :, 0:2], in1=rzhv,
                                        op=mult)
                ld_b.dma_start(out=ot.ap()[:, so * 2:(so + sls) * 2],
                               in_=otile[:])

            # 2-stage software pipeline: the mult->s1->a->recip->fmul chain
            # spans more than one slab period, so slab o's mid runs after
            # head[o+1] and its tail after head[o+2] — no in-order engine
            # queue ever waits across slabs
            stages = []
            so = 0
            for o, sls in enumerate(SLABS):
                stages.append(head(o, sls, so))
                if o >= 1:
                    stages[o - 1] = mid(stages[o - 1])
                if o >= 2:
                    tail(stages[o - 2])
                so += sls
            n = len(SLABS)
            stages[n - 1] = mid(stages[n - 1])
            tail(stages[n - 2])
            tail(stages[n - 1])
    nc.compile()
    return nc


def get_nc():
    if "nc" not in _CACHE:
        _CACHE["nc"] = _build()
    return _CACHE["nc"]


def make_in_maps(tMP, tKF, kf_rows, mp_rows):
    """Pack measurements into pose-cells; returns per-core inputs + slot maps."""
    T = np.asarray(tKF, dtype=np.float32)
    A = np.empty((N_KF, 3, 4), np.float32)
    A[:, 0] = FX * T[:, 0] + CX * T[:, 2]
    A[:, 1] = FY * T[:, 1] + CY * T[:, 2]
    A[:, 2] = T[:, 2]
    # row layout per cell: [A(i,j01) pairs i-major (6)] [A(:,2) (3)] [A(:,3) (3)]
    A12 = np.concatenate([A[:, :, 0:2].reshape(N_KF, 6),
                          A[:, :, 2], A[:, :, 3]], axis=1).astype(np.float16)
    empty_row = np.zeros(12, np.float16)
    empty_row[11] = 1.0        # a2 = 1 for padding cells -> out = 0, no NaN
    tMP = np.asarray(tMP, dtype=np.float32)
    # h stream per slot: (x, y, z, z, z) - z replicated so the z-products
    # read a packed lane per i
    homo = np.empty((N_MP, 5), np.float32)
    homo[:, 0:2] = tMP[:, 0:2]
    homo[:, 2:5] = tMP[:, 2:3]
    homo = homo.astype(np.float16)
    in_maps = []
    slot_maps = []
    for c in range(N_CORES):
        kf = kf_rows[c * MC:(c + 1) * MC]
        mp = mp_rows[c * MC:(c + 1) * MC]
        counts = np.bincount(kf, minlength=N_KF)
        ncells_k = -(-counts // S)
        cell_off = np.concatenate([[0], np.cumsum(ncells_k)])
        ncells = int(cell_off[-1])
        assert ncells <= P * CH, f"cell overflow: {ncells} > {P * CH}"
        order = np.argsort(kf, kind="stable")
        kfs = kf[order]
        starts = np.concatenate([[0], np.cumsum(counts)])
        j = np.arange(MC, dtype=np.int64) - starts[kfs]
        slot = (cell_off[kfs] + j // S) * S + (j % S)    # flat in [0, TOT)
        hpa = np.zeros((TOT, 5), np.float16)
        hpa[slot] = homo[mp[order]]
        kcell = np.repeat(np.arange(N_KF), ncells_k)
        tbl = np.empty((P * CH, 12), np.float16)
        tbl[:ncells] = A12[kcell]
        tbl[ncells:] = empty_row
        # interleave per slab: [table cells | h slots] so each slab is one DMA
        tbl_p = tbl.reshape(P, CH, 12)
        hpa_p = hpa.reshape(P, SLOTS, 5)
        blocks = []
        soff = 0
        for sls in SLABS:
            c0, s0 = soff // S, soff
            blocks.append(tbl_p[:, c0:c0 + sls // S].reshape(P, sls // S * 12))
            blocks.append(hpa_p[:, s0:s0 + sls].reshape(P, sls * 5))
            soff += sls
        in_maps.append({"hb": np.concatenate(blocks, axis=1)})
        slot_maps.append((order, slot))
    return in_maps, slot_maps


def assemble(results, slot_maps):
    outs = []
    for c in range(N_CORES):
        o = np.asarray(results[c]["ot"]).reshape(TOT, 2)
        order, slot = slot_maps[c]
        r = np.empty((MC, 2), np.float32)
        r[order] = o[slot].astype(np.float32)
        outs.append(r)
    return np.concatenate(outs, axis=0)


def kernel(tMP, tKF, idxKF, idxMP, meas_kf, meas_mp):
    import time

    from concourse.bass_utils import run_bass_kernel_spmd

    nc = get_nc()
    # id -> row resolution (identity for sorted arange id tables)
    kf_rows = np.searchsorted(np.asarray(idxKF), np.asarray(meas_kf)).astype(np.int64)
    mp_rows = np.searchsorted(np.asarray(idxMP), np.asarray(meas_mp)).astype(np.int64)
    in_maps, slot_maps = make_in_maps(np.asarray(tMP), np.asarray(tKF),
                                      kf_rows, mp_rows)
    try:
        res = run_bass_kernel_spmd(nc, in_maps, core_ids=list(range(N_CORES)))
    except Exception:
        # transient NRT exec-unit errors have been observed when a previous
        # process was still draining the cores; one retry recovers them
        time.sleep(2.0)
        res = run_bass_kernel_spmd(nc, in_maps, core_ids=list(range(N_CORES)))
    return assemble(res.results, slot_maps)


# revision 38
# speedup vs baseline: 1.0386x; 1.0029x over previous
"""Trainium2 Bass kernel for nn_BAGDnet (gnn_message_passing).

Computation (per measurement m):
    T = tKF[meas_kf[m]]          # 4x4 pose
    p = tMP[meas_mp[m]]          # 3d map point
    pts = T[:3] @ [p, 1]
    out[m] = (pts0/pts2*FX + CX, pts1/pts2*FY + CY)

idxKF / idxMP are sorted unique arange id tables, so searchsorted(idx, meas)
== meas and measurement ids index the tables directly.

Sharding strategy (data-parallel over M per the hint): 2M measurements split
across 8 cores. Per core, measurements are grouped by pose into fixed-size
cells (S=4 slots, one pose per cell, poses spanning multiple cells get their
table row duplicated), laid out as 128 partitions x 502 cells. The pose rows
are pre-projected on host into A = [FX*T0+CX*T2; FY*T1+CY*T2; T2] so the
device computes out = (A0.h/A2.h, A1.h/A2.h) with no epilogue add.

On device the pose row for a cell is never materialized per measurement:
the multiply reads the 12-value row straight from a tiny per-partition table
through a stride-0 broadcast access pattern. This cuts HBM traffic from
68 B/meas (gathered-pose streaming) to ~15 B/meas (fp16 h-vector + fp16 out
+ table), turning the kernel from DMA-bound into engine-balanced:
  DVE    : product m = A (*) h (fp16 TensorTensor 2x mode), part of the
           first adds, and the final multiply by the reciprocal
  GPSIMD : rest of the first adds + the second adds
  ACT    : the perspective reciprocal (table func, duplicated fp16 lanes)
Host gathers the points into cell order (id->row is identity here), and
un-permutes the fp16 device output back to measurement order in f32.
"""

import numpy as np

M = 2_000_000
N_KF = 2_000
N_MP = 200_000
N_CORES = 8
MC = M // N_CORES          # 250_000 measurements per core
P = 128
S = 4                      # slots per cell (one pose per cell)
CH = 502                   # cells per partition
SLOTS = CH * S             # 2008 slots per partition
TOT = P * SLOTS            # 257024 slots per core (~2.8% padding)
# small head slab starts compute sooner (first-load latency is mostly DMA
# pipeline constants + transfer); small tail slab shortens the last
# a->recip->fmul->store chain after the bulk compute ends
SLABS = [64, 192, 288, 320, 320, 320, 312, 128, 64]
assert sum(SLABS) == SLOTS and all(s % S == 0 for s in SLABS)
FX = 320.0
FY = 320.0
CX = 320.0
CY = 240.0

_CACHE = {}


def _act_recip(nc, mybir, out, in_):
    """Scalar-engine reciprocal: out = 1 / in_.

    Emitted directly (the bass wrapper refuses ActivationFunctionType.
    Reciprocal out of fp32-training accuracy caution; the act-table func is
    plenty accurate for this kernel's 2e-2 tolerance)."""
    se = nc.scalar
    ins = [se.lower_ap(in_)]
    for v in (0.0, 1.0, 0.0):      # bias, scale, alpha immediates
        ins.append(mybir.ImmediateValue(dtype=mybir.dt.float32, value=v))
    return se.add_instruction(
        mybir.InstActivation(
            name=se.bass.get_next_instruction_name(),
            func=mybir.ActivationFunctionType.Reciprocal,
            ins=ins,
            outs=[se.lower_ap(out)],
        )
    )


def _build():
    import concourse.bacc as bacc
    import concourse.mybir as mybir
    import concourse.tile as tile

    f16 = mybir.dt.float16
    f32 = mybir.dt.float32
    mult, add = mybir.AluOpType.mult, mybir.AluOpType.add

    nc = bacc.Bacc("TRN2", target_bir_lowering=False, debug=False)
    # per-slab [table-cells | h-slots] packed into ONE stream: one DMA per
    # slab (instead of two) halves the serialized HWDGE issue slots
    TOTB = CH * 12 + SLOTS * 5
    hb = nc.dram_tensor("hb", [P, TOTB], f16, kind="ExternalInput")
    ot = nc.dram_tensor("ot", [P, SLOTS * 2], f16, kind="ExternalOutput")

    with tile.TileContext(nc) as tc:
        with tc.tile_pool(name="hpool", bufs=4) as hpool, \
             tc.tile_pool(name="tpool", bufs=4) as tpool, \
             tc.tile_pool(name="mpool", bufs=4) as mpool, \
             tc.tile_pool(name="spool", bufs=4) as spool, \
             tc.tile_pool(name="apool", bufs=4) as apool, \
             tc.tile_pool(name="opool", bufs=4) as opool:
            def head(o, sls, so):
                """Slab front: loads, products, 2x-mode adds. Returns state."""
                chs = sls // S
                bo = so * 5 + (so // S) * 12     # slab offset in hb
                sz = chs * 12 + sls * 5
                ld_b = nc.sync
                ct = hpool.tile([P, sz], f16, tag="ct")
                ld_b.dma_start(out=ct[:], in_=hb.ap()[:, bo:bo + sz])
                tt_ap = ct[:][:, 0:chs * 12]
                ht_ap = ct[:][:, chs * 12:sz]
                # h stream per slot is (x, y, z, z, z); table row per cell is
                # [A(i,j01) pairs (6)] [A(i,2) (3)] [A(i,3) (3)] so that every
                # operand below has a packed (stride-1) last dim -> DVE 2x.
                # mA[p, cell, s, i, k] = A[cell, i, k] * h[cell, s, k], k=x,y
                mA = mpool.tile([P, sls * 6], f16, tag="mA")
                t_pair = tt_ap.rearrange("p (seg o i k) -> p seg o i k",
                                         seg=chs, o=1, i=6, k=2)[:, :, :, 0:3] \
                              .to_broadcast([P, chs, S, 3, 2])
                h_xy = ht_ap.rearrange("p (seg s o c) -> p seg s o c",
                                       seg=chs, s=S, o=1, c=5)[:, :, :, :, 0:2] \
                            .to_broadcast([P, chs, S, 3, 2])
                mA_v = mA[:].rearrange("p (seg s i k) -> p seg s i k",
                                       seg=chs, s=S, i=3, k=2)
                nc.vector.tensor_tensor(out=mA_v, in0=h_xy, in1=t_pair, op=mult)
                # mB[p, cell, s, i] = A[cell, i, 2] * z   (z replicated in h)
                mB = mpool.tile([P, sls * 3], f16, tag="mB")
                t_z = tt_ap.rearrange("p (seg o i) -> p seg o i",
                                      seg=chs, o=1, i=12)[:, :, :, 6:9] \
                           .to_broadcast([P, chs, S, 3])
                h_zzz = ht_ap.rearrange("p (seg s c) -> p seg s c",
                                        seg=chs, s=S, c=5)[:, :, :, 2:5]
                mB_v = mB[:].rearrange("p (seg s i) -> p seg s i",
                                       seg=chs, s=S, i=3)
                nc.vector.tensor_tensor(out=mB_v, in0=h_zzz, in1=t_z, op=mult)
                # s1b[p, sl, i] = mB + A[cell, i, 3]   (translation fold, 2x)
                s1b = spool.tile([P, sls * 3], f16, tag="s1b")
                s1b_v = s1b[:].rearrange("p (seg s i) -> p seg s i",
                                         seg=chs, s=S, i=3)
                t_c = tt_ap.rearrange("p (seg o i) -> p seg o i",
                                      seg=chs, o=1, i=12)[:, :, :, 9:12] \
                           .to_broadcast([P, chs, S, 3])
                nc.vector.tensor_tensor(out=s1b_v, in0=mB_v, in1=t_c, op=add)
                # s1a[p, sl, i] = mA[.., 0] + mA[.., 1]  (strided; gpsimd)
                mAs = mA[:].rearrange("p (sl i k) -> p sl i k", i=3, k=2)
                s1a = spool.tile([P, sls * 3], f16, tag="s1a")
                s1a_v = s1a[:].rearrange("p (sl i) -> p sl i", i=3)
                nc.gpsimd.tensor_tensor(out=s1a_v, in0=mAs[:, :, :, 0],
                                        in1=mAs[:, :, :, 1], op=add)
                return (sls, so, s1a[:].rearrange("p (sl i) -> p sl i", i=3),
                        s1b[:].rearrange("p (sl i) -> p sl i", i=3), ld_b)

            def mid(st):
                """Slab middle: final adds (split DVE/gpsimd) + reciprocal."""
                sls, so, s1a_v, s1b_v, ld_b = st
                k2 = (sls * 39 // 64) // 4 * 4   # a slots on DVE, rest gpsimd
                # a[p, sl, i] = s1a + s1b   (both packed fp16 -> DVE 2x)
                a = apool.tile([P, sls * 3], f16, tag="a")
                av = a[:].rearrange("p (sl i) -> p sl i", i=3)
                nc.vector.tensor_tensor(out=av[:, 0:k2], in0=s1a_v[:, 0:k2],
                                        in1=s1b_v[:, 0:k2], op=add)
                nc.gpsimd.tensor_tensor(out=av[:, k2:sls], in0=s1a_v[:, k2:sls],
                                        in1=s1b_v[:, k2:sls], op=add)
                # rzh[p, sl, c] = 1/a2 duplicated into two packed fp16 lanes
                # (scalar engine reciprocal; interp-exact, z in [3,7] is well
                # inside the +-[2^-42, 2^42] valid range)
                rzh = apool.tile([P, sls * 2], f16, tag="rzh")
                rzhv = rzh[:].rearrange("p (sl c) -> p sl c", c=2)
                _act_recip(nc, mybir, out=rzhv,
                           in_=av[:, :, 2:3].to_broadcast([P, sls, 2]))
                return (sls, so, av, rzhv, ld_b)

            def tail(st):
                """Slab back: perspective multiply + store."""
                sls, so, av, rzhv, ld_b = st
                # out = a01 * rzh   (DVE, 2x: all packed fp16)
                otile = opool.tile([P, sls * 2], f16, tag="ot")
                ov = otile[:].rearrange("p (sl c) -> p sl c", c=2)
                nc.vector.tensor_tensor(out=ov, in0=av[:, :, 0:2], in1=rzhv,
                                        op=mult)
                ld_b.dma_start(out=ot.ap()[:, so * 2:(so + sls) * 2],
                               in_=otile[:])

            # 2-stage software pipeline: the mult->s1->a->recip->fmul chain
            # spans more than one slab period, so slab o's mid runs after
            # head[o+1] and its tail after head[o+2] — no in-order engine
            # queue ever waits across slabs
            stages = []
            so = 0
            for o, sls in enumerate(SLABS):
                stages.append(head(o, sls, so))
                if o >= 1:
                    stages[o - 1] = mid(stages[o - 1])
                if o >= 2:
                    tail(stages[o - 2])
                so += sls
            n = len(SLABS)
            stages[n - 1] = mid(stages[n - 1])
            tail(stages[n - 2])
            tail(stages[n - 1])
    nc.compile()
    return nc


def get_nc():
    if "nc" not in _CACHE:
        _CACHE["nc"] = _build()
    return _CACHE["nc"]


def make_in_maps(tMP, tKF, kf_rows, mp_rows):
    """Pack measurements into pose-cells; returns per-core inputs + slot maps."""
    T = np.asarray(tKF, dtype=np.float32)
    A = np.empty((N_KF, 3, 4), np.float32)
    A[:, 0] = FX * T[:, 0] + CX * T[:, 2]
    A[:, 1] = FY * T[:, 1] + CY * T[:, 2]
    A[:, 2] = T[:, 2]
    # row layout per cell: [A(i,j01) pairs i-major (6)] [A(:,2) (3)] [A(:,3) (3)]
    A12 = np.concatenate([A[:, :, 0:2].reshape(N_KF, 6),
                          A[:, :, 2], A[:, :, 3]], axis=1).astype(np.float16)
    empty_row = np.zeros(12, np.float16)
    empty_row[11] = 1.0        # a2 = 1 for padding cells -> out = 0, no NaN
    tMP = np.asarray(tMP, dtype=np.float32)
    # h stream per slot: (x, y, z, z, z) - z replicated so the z-products
    # read a packed lane per i
    homo = np.empty((N_MP, 5), np.float32)
    homo[:, 0:2] = tMP[:, 0:2]
    homo[:, 2:5] = tMP[:, 2:3]
    homo = homo.astype(np.float16)
    in_maps = []
    slot_maps = []
    for c in range(N_CORES):
        kf = kf_rows[c * MC:(c + 1) * MC]
        mp = mp_rows[c * MC:(c + 1) * MC]
        counts = np.bincount(kf, minlength=N_KF)
        ncells_k = -(-counts // S)
        cell_off = np.concatenate([[0], np.cumsum(ncells_k)])
        ncells = int(cell_off[-1])
        assert ncells <= P * CH, f"cell overflow: {ncells} > {P * CH}"
        order = np.argsort(kf, kind="stable")
        kfs = kf[order]
        starts = np.concatenate([[0], np.cumsum(counts)])
        j = np.arange(MC, dtype=np.int64) - starts[kfs]
        slot = (cell_off[kfs] + j // S) * S + (j % S)    # flat in [0, TOT)
        hpa = np.zeros((TOT, 5), np.float16)
        hpa[slot] = homo[mp[order]]
        kcell = np.repeat(np.arange(N_KF), ncells_k)
        tbl = np.empty((P * CH, 12), np.float16)
        tbl[:ncells] = A12[kcell]
        tbl[ncells:] = empty_row
        # interleave per slab: [table cells | h slots] so each slab is one DMA
        tbl_p = tbl.reshape(P, CH, 12)
        hpa_p = hpa.reshape(P, SLOTS, 5)
        blocks = []
        soff = 0
        for sls in SLABS:
            c0, s0 = soff // S, soff
            blocks.append(tbl_p[:, c0:c0 + sls // S].reshape(P, sls // S * 12))
            blocks.append(hpa_p[:, s0:s0 + sls].reshape(P, sls * 5))
            soff += sls
        in_maps.append({"hb": np.concatenate(blocks, axis=1)})
        slot_maps.append((order, slot))
    return in_maps, slot_maps


def assemble(results, slot_maps):
    outs = []
    for c in range(N_CORES):
        o = np.asarray(results[c]["ot"]).reshape(TOT, 2)
        order, slot = slot_maps[c]
        r = np.empty((MC, 2), np.float32)
        r[order] = o[slot].astype(np.float32)
        outs.append(r)
    return np.concatenate(outs, axis=0)


def kernel(tMP, tKF, idxKF, idxMP, meas_kf, meas_mp):
    import time

    from concourse.bass_utils import run_bass_kernel_spmd

    nc = get_nc()
    # id -> row resolution (identity for sorted arange id tables)
    kf_rows = np.searchsorted(np.asarray(idxKF), np.asarray(meas_kf)).astype(np.int64)
    mp_rows = np.searchsorted(np.asarray(idxMP), np.asarray(meas_mp)).astype(np.int64)
    in_maps, slot_maps = make_in_maps(np.asarray(tMP), np.asarray(tKF),
                                      kf_rows, mp_rows)
    try:
        res = run_bass_kernel_spmd(nc, in_maps, core_ids=list(range(N_CORES)))
    except Exception:
        # transient NRT exec-unit errors have been observed when a previous
        # process was still draining the cores; one retry recovers them
        time.sleep(2.0)
        res = run_bass_kernel_spmd(nc, in_maps, core_ids=list(range(N_CORES)))
    return assemble(res.results, slot_maps)
